# revision 16
# baseline (speedup 1.0000x reference)
"""ChannelMerger TRN2 kernel.

Math (per batch b):
  emb[c,d] = fourier embedding of positions[b,c] (cos block d<1024, sin block d>=1024)
  scores[c,o] = sum_d emb[c,d] * heads[o,d]  (+ -inf on invalid channels)
  w = softmax_c(scores);  out[o,t] = sum_c x[b,c,t] * w[c,o]

Device strategy (data-parallel over B across 8 cores, 4 batches/core):
  * Half-angle identity: cos(L) = 1-2*sin^2(L/2), sin(L) = 1-2*sin^2(L/2 - pi/4).
    Host reduces per-term phases mod pi into [-pi/2, pi/2) so the on-device
    Sin argument h = u+v is always in [-pi, pi) - the ACT Sin spline's exact
    valid domain. No on-device range reduction needed.
  * h is built by a K=128 fp16 matmul: 0/1 selector (lhsT) x per-(term,c)
    phase rows hi/lo-split in fp16 (exact-to-~4e-7 sum in fp32 PSUM).
  * q = sin^2(h) via ACT Sin + DVE square -> fp16.
  * The affine emb = 1-2q folds into the scores matmul:
      16*scores = sum_d q_d * (-32*heads_d) + 16*rowsum(heads)   (extra K=1 row)
    and the 1/16 un-scale folds into Exp's free affine.
  * Invalid-channel -inf offset rides Exp's per-partition bias (-50 => exp ~ 0).
  * e = exp(scores) in [C,O] layout; s[o] = column sums via ones-matmul;
    1/s folds into the PSUM->SBUF eviction scale of the final matmul.
  * All ACT Sin ops are ordered before all Exp ops (one table set load each).
"""
import os
import numpy as np

import concourse.bacc as bacc
import concourse.tile as tile
from concourse import mybir
from concourse.bass_utils import run_bass_kernel_spmd
from concourse.tile import add_dep_helper

F32 = mybir.dt.float32
F16 = mybir.dt.float16

B, C, T, O, D = 32, 512, 2048, 256, 2048
NCORES = 8
BS = B // NCORES          # batches per core
NF = 32                   # n_freqs
DP = NF * NF              # 1024 phase rows (per cos/sin block)
MARGIN, INVALID = 0.2, -0.1
PI = float(np.pi)
HSCALE = 16.0             # scores are computed scaled by 16 (fp16 headroom)

SinF = mybir.ActivationFunctionType.Sin
ExpF = mybir.ActivationFunctionType.Exp
CopyF = mybir.ActivationFunctionType.Copy


def _build():
    nc = bacc.Bacc("TRN2", target_bir_lowering=False, debug=False)

    x_s = nc.dram_tensor("x_s", [BS, C, T], F16, kind="ExternalInput")
    rhs_all = nc.dram_tensor("rhs_all", [128, 2 * BS, C], F16,
                             kind="ExternalInput")
    fsel = nc.dram_tensor("fsel", [128, DP], F16, kind="ExternalInput")
    ht2 = nc.dram_tensor("ht2", [128, D // 128, O], F16,
                         kind="ExternalInput")
    h1r = nc.dram_tensor("h1r", [1, O], F16, kind="ExternalInput")
    selones = nc.dram_tensor("selones", [1, C], F16, kind="ExternalInput")
    offs = nc.dram_tensor("offs", [128, BS * (C // 128)], F32,
                          kind="ExternalInput")
    out_d = nc.dram_tensor("out_s", [BS, O, T], F32, kind="ExternalOutput")

    NKT = D // 128        # 16 contraction tiles for scores
    NCT = C // 128        # 4 channel tiles
    NTT = T // 512        # 4 time tiles
    NOH = O // 128        # 2 output-head halves

    with tile.TileContext(nc) as tc:
        with (
            tc.tile_pool(name="singles", bufs=1) as sg,
            tc.tile_pool(name="tp", bufs=3) as tp,
            tc.tile_pool(name="qp", bufs=26) as qp,
            tc.tile_pool(name="sr", bufs=1) as sr,
            tc.tile_pool(name="ep", bufs=1) as ep,
            tc.tile_pool(name="xp", bufs=16) as xp,
            tc.tile_pool(name="op", bufs=4) as op,
        ):
            ps_head = tc.tile_pool(name="ps_head", bufs=3, space="PSUM")
            ps_loc = ps_head.__enter__()
            ps_sc_pool = tc.tile_pool(name="ps_sc", bufs=2, space="PSUM")
            ps_sc = ps_sc_pool.__enter__()
            # ---- constants / weights (critical-path first: fsel + b0 rhs) ----
            fsel_sb = sg.tile([128, DP], F16, tag="fsel")
            nc.sync.dma_start(fsel_sb[:], fsel[:])
            rhs_bank = sg.tile([128, 2 * BS, C], F16, tag="rhs")
            nc.sync.dma_start(rhs_bank[:, 0:1], rhs_all[:, 0:1])
            nc.sync.dma_start(rhs_bank[:, BS:BS + 1], rhs_all[:, BS:BS + 1])
            nc.sync.dma_start(rhs_bank[:, 1:BS], rhs_all[:, 1:BS])
            nc.sync.dma_start(rhs_bank[:, BS + 1:], rhs_all[:, BS + 1:])
            rhs_sb = {}
            for b in range(BS):
                for blk in range(2):
                    rhs_sb[b, blk] = rhs_bank[:, blk * BS + b]
            ht2_sb = sg.tile([128, NKT, O], F16, tag="ht2")
            nc.sync.dma_start(ht2_sb[:], ht2[:])
            h1_sb = sg.tile([1, O], F16, tag="h1")
            nc.sync.dma_start(h1_sb[:], h1r[:])
            ones_sb = sg.tile([1, C], F16, tag="ones")
            nc.sync.dma_start(ones_sb[:], selones[:])
            ones128 = sg.tile([128, 1], F16, tag="ones128")
            nc.vector.memset(ones128[:], 1.0)
            offs_bank = sg.tile([128, BS * NCT], F32, tag="offs")
            nc.sync.dma_start(offs_bank[:], offs[:])
            offs_sb = {}
            for b in range(BS):
                for ct in range(NCT):
                    offs_sb[b, ct] = offs_bank[:, b * NCT + ct:b * NCT + ct + 1]

            # ---- phase A: embedding q tiles (kt pairs -> wide ops) ----
            sin_insts = []
            q = {}
            wi = 0
            for b in range(BS):
                for blk in range(2):           # 0: cos rows, 1: sin rows
                    for dt2 in range(4):
                        ph = ps_loc.tile([128, 2, C], F32, tag="ph")
                        for half in range(2):
                            dt = dt2 * 2 + half
                            nc.tensor.matmul(
                                ph[:, half], fsel_sb[:, dt * 128:(dt + 1) * 128],
                                rhs_sb[b, blk][:], start=True, stop=True)
                        ts_ = tp.tile([128, 2, C], F32, tag="t")
                        si = nc.scalar.activation(ts_[:], ph[:], SinF)
                        sin_insts.append(si)
                        qt = qp.tile([128, 2, C], F16, tag="q")
                        if wi % 3 == 2:
                            nc.gpsimd.tensor_mul(qt[:], ts_[:], ts_[:])
                        else:
                            nc.vector.tensor_mul(qt[:], ts_[:], ts_[:])
                        wi += 1
                        q[b, blk * 8 + dt2 * 2] = qt[:, 0]
                        q[b, blk * 8 + dt2 * 2 + 1] = qt[:, 1]

            # ---- phase B: scores (x16) -> raw SBUF ----
            sraw = {}
            for b in range(BS):
                for ct in range(NCT):
                    psc = ps_sc.tile([128, O], F32, tag="psc")
                    for kt in range(NKT):
                        nc.tensor.matmul(
                            psc[:], q[b, kt][:, ct * 128:(ct + 1) * 128],
                            ht2_sb[:, kt], start=(kt == 0), stop=False)
                    nc.tensor.matmul(
                        psc[:], ones_sb[:, ct * 128:(ct + 1) * 128], h1_sb[:],
                        start=False, stop=True)
                    st = sr.tile([128, O], F32, tag=f"sraw{b}_{ct}")
                    nc.vector.tensor_copy(st[:], psc[:])
                    sraw[b, ct] = st

            # ---- phase C: exp (after ALL sins: single table-set switch) ----
            e = {}
            last_sin = sin_insts[-1]
            for b in range(BS):
                for ct in range(NCT):
                    et = ep.tile([128, O], F16, tag=f"e{b}_{ct}")
                    ei = nc.scalar.activation(
                        et[:], sraw[b, ct][:], ExpF,
                        bias=offs_sb[b, ct][:], scale=1.0 / HSCALE)
                    add_dep_helper(ei.ins, last_sin.ins, sync=False,
                                   reason="keep Sin/Exp ACT table sets phased")
                    e[b, ct] = et

            ps_sc_pool.__exit__(None, None, None)
            ps_head.__exit__(None, None, None)
            ps_ss_pool = tc.tile_pool(name="ps_ss", bufs=1, space="PSUM")
            ps_ss = ps_ss_pool.__enter__()
            ps_o_pool = tc.tile_pool(name="ps_o", bufs=4, space="PSUM")
            ps_o = ps_o_pool.__enter__()

            # ---- phase D: softmax denominators ----
            sinv = {}
            for b in range(BS):
                for oh in range(NOH):
                    pss = ps_ss.tile([128, 1], F32, tag="pss")
                    for ct in range(NCT):
                        nc.tensor.matmul(
                            pss[:], e[b, ct][:, oh * 128:(oh + 1) * 128],
                            ones128[:], start=(ct == 0), stop=(ct == NCT - 1))
                    sv = sg.tile([128, 1], F32, tag=f"sv{b}_{oh}")
                    nc.vector.reciprocal(sv[:], pss[:])
                    sinv[b, oh] = sv

            # ---- phase E: weighted sum + normalized eviction ----
            x_v = x_s.rearrange("b (ct k) t -> b ct k t", k=128)
            for b in range(BS):
                xt = []
                for ct in range(NCT):
                    xtile = xp.tile([128, T], F16, tag="x")
                    nc.sync.dma_start(xtile[:], x_v[b, ct])
                    xt.append(xtile)
                for tt in range(NTT):
                    for oh in range(NOH):
                        po = ps_o.tile([128, 512], F32, tag="po")
                        for ct in range(NCT):
                            nc.tensor.matmul(
                                po[:], e[b, ct][:, oh * 128:(oh + 1) * 128],
                                xt[ct][:, tt * 512:(tt + 1) * 512],
                                start=(ct == 0), stop=(ct == NCT - 1))
                        ot = op.tile([128, 512], F32, tag="o")
                        if oh == 0:
                            nc.scalar.activation(ot[:], po[:], CopyF,
                                                 scale=sinv[b, oh][:])
                        else:
                            nc.vector.tensor_scalar_mul(ot[:], po[:],
                                                        sinv[b, oh][:])
                        nc.sync.dma_start(
                            out_d[b, oh * 128:(oh + 1) * 128,
                                  tt * 512:(tt + 1) * 512], ot[:])
            ps_o_pool.__exit__(None, None, None)
            ps_ss_pool.__exit__(None, None, None)

    nc.compile()
    return nc


def _host_prep(x, positions, heads):
    """Build per-core input maps."""
    x = np.asarray(x)
    positions = np.asarray(positions, np.float32)
    heads = np.asarray(heads, np.float32)

    # phases, in float64: half-angle per-term reductions mod pi -> [-pi/2, pi/2)
    qxy = (positions.astype(np.float64) + MARGIN) / (1.0 + 2.0 * MARGIN)
    qx, qy = qxy[..., 0], qxy[..., 1]          # [B, C]
    i = np.arange(NF, dtype=np.float64)[None, :, None]   # [1, 32, 1]
    wx = i * qx[:, None, :]                    # [B, 32, C]
    wy = i * qy[:, None, :]
    wxs = wx - 0.25                            # sin block: extra -pi/4 half-phase

    def red(w):                                # pi*(w - round(w)) in [-pi/2, pi/2)
        return PI * (w - np.round(w))

    def hilo(u):                               # fp16 hi/lo split of float64 angles
        hi = u.astype(np.float16)
        lo = (u - hi.astype(np.float64)).astype(np.float16)
        return hi, lo

    uc_hi, uc_lo = hilo(red(wx))
    us_hi, us_lo = hilo(red(wxs))
    v_hi, v_lo = hilo(red(wy))

    def pack(uhi, ulo):                        # [B, 128, C]
        return np.concatenate([uhi, v_hi, ulo, v_lo], axis=1)

    rhs_c = pack(uc_hi, uc_lo)
    rhs_s = pack(us_hi, us_lo)

    # selector [128, 1024]: column d'=(i,j) reads u[i], v[j] (hi and lo)
    fsel = np.zeros((128, DP), np.float16)
    dp = np.arange(DP)
    ii, jj = dp // NF, dp % NF
    fsel[ii, dp] = 1.0
    fsel[NF + jj, dp] = 1.0
    fsel[2 * NF + ii, dp] = 1.0
    fsel[3 * NF + jj, dp] = 1.0

    ht2_flat = ((-2.0 * HSCALE) * heads.T.astype(np.float64)).astype(np.float16)
    ht2 = np.ascontiguousarray(
        ht2_flat.reshape(D // 128, 128, O).transpose(1, 0, 2))  # [128, 16, O]
    h1r = (HSCALE * heads.astype(np.float64).sum(axis=1))[None, :].astype(
        np.float16)                                                    # [1, O]
    selones = np.ones((1, C), np.float16)

    invalid = np.all(positions == INVALID, axis=-1)                    # [B, C]
    offs = np.where(invalid, np.float32(-50.0), 0.0).astype(np.float32)
    # bias is applied AFTER the 1/16 un-scale, so -50 is the true offset
    x16 = x.astype(np.float16)

    in_maps = []
    for core in range(NCORES):
        sl = slice(core * BS, (core + 1) * BS)
        rhs_core = np.concatenate([rhs_c[sl], rhs_s[sl]], axis=0
                                  ).transpose(1, 0, 2)           # [128,2*BS,C]
        offs_core = offs[sl].reshape(BS * 4, 128).T                # [128, BS*4]
        in_maps.append(dict(
            x_s=np.ascontiguousarray(x16[sl]),
            rhs_all=np.ascontiguousarray(rhs_core.astype(np.float16)),
            fsel=fsel,
            ht2=ht2,
            h1r=h1r,
            selones=selones,
            offs=np.ascontiguousarray(offs_core),
        ))
    return in_maps


_NC_CACHE = None
last_exec_time_ns = None
last_profile = None


def _install_ntff_shim():
    """Register an antenv.axon_hooks NTFF profile hook via ctypes against
    libaxon_pjrt.so (the agent image lacks the shim module). Trace-only."""
    import importlib.util
    if importlib.util.find_spec("antenv") is None:
        return False
    try:
        from antenv.axon_hooks import get_axon_ntff_profile_hook  # noqa: F401
        return True
    except ImportError:
        pass
    import sys
    import types
    import ctypes
    import contextlib
    so_path = "/opt/axon/libaxon_pjrt.so"
    if not os.path.exists(so_path):
        return False
    lib = ctypes.CDLL(so_path)
    if not hasattr(lib, "axon_start_nrt_profile"):
        return False
    lib.axon_start_nrt_profile.argtypes = [ctypes.POINTER(ctypes.c_int64),
                                           ctypes.c_size_t]
    lib.axon_start_nrt_profile.restype = ctypes.c_int64
    lib.axon_stop_nrt_profile.argtypes = [ctypes.c_char_p]
    lib.axon_stop_nrt_profile.restype = ctypes.c_int64

    @contextlib.contextmanager
    def _hook(output_dir, device_ids):
        import jax
        jax.devices()
        if device_ids:
            ids = (ctypes.c_int64 * len(device_ids))(*device_ids)
            rc = lib.axon_start_nrt_profile(ids, len(device_ids))
        else:
            rc = lib.axon_start_nrt_profile(None, 0)
        if rc != 0:
            raise RuntimeError(f"axon_start_nrt_profile rc={rc}")
        try:
            yield
        finally:
            n = lib.axon_stop_nrt_profile(str(output_dir).encode())
            print(f"ntff profile: {n} file(s) written to {output_dir}")

    import antenv
    mod = types.ModuleType("antenv.axon_hooks")
    holder = {"h": _hook}
    mod.get_axon_ntff_profile_hook = lambda: holder["h"]
    mod.set_axon_ntff_profile_hook = lambda h: holder.__setitem__("h", h)
    sys.modules["antenv.axon_hooks"] = mod
    antenv.axon_hooks = mod
    return True


def kernel(x, positions, heads):
    global _NC_CACHE, last_exec_time_ns, last_profile
    if _NC_CACHE is None:
        _NC_CACHE = _build()
    nc = _NC_CACHE
    in_maps = _host_prep(x, positions, heads)
    trace = os.environ.get("KERNEL_TRACE", "0") == "1"
    kwargs = {}
    if trace:
        trace = _install_ntff_shim()
    if trace:
        import concourse.bass_utils as _bu
        _bu.upload_artifacts = lambda d: d          # no artifact share here
        tdir = os.environ.get("KERNEL_TRACE_DIR")
        if tdir:
            os.makedirs(tdir, exist_ok=True)
            kwargs["tmpdir"] = tdir
        kwargs["trace_cores"] = [0]
    res = run_bass_kernel_spmd(nc, in_maps, list(range(NCORES)), trace=trace,
                               **kwargs)
    last_exec_time_ns = res.exec_time_ns
    last_profile = res.profile_json
    out = np.concatenate([r["out_s"] for r in res.results], axis=0)
    return out.astype(np.float32)
